# revision 4
# baseline (speedup 1.0000x reference)
"""Gaussian histogram kernel for TRN2, 8 NeuronCores, data-parallel over points.

Math (per point n, bin b):
  r0 = ||means_n - scan_point||, sigma = max(exp(pas_n), hb), hb = BIN_RES/2
  g = exp(-(r_b - r0)^2 / (2 sigma^2))
  pr = clip(hb * (coeff*pdf1 + (1-coeff)*pdf2), 0, 1)
     = g * beta * relu(r_b - thr)          [upper clip never binds: hb<=sigma]
       beta = hb*(1-coeff)/sigma^2, thr = r0 - coeff*c1*sigma/(1-coeff)
  hist_b = sum_n I_n * pr_{n,b} / r_b^2,   I_n = sigmoid(op_n)*col_n^2

On-chip mapping per core (16384 points = 128 tiles of 128 partitions):
  g~  = DerivativeErf(s*R + bias)  (ScalarE LUT; = 2/sqrt(pi) * g)
  hinge = max(R - thr, 0)          (VectorE dual-op tensor_scalar, 2x fp32)
  pp  = g~ * hinge                 (VectorE tensor_tensor)
  PSUM[1,512] += w^T @ pp          (TensorE, w = I*beta*sqrt(pi)/2)
Partial [512] per core; host sums the 8 partials (the all-reduce/unshard).
"""
import numpy as np

import concourse.bacc as bacc
import concourse.mybir as mybir
from concourse.tile import TileContext
from concourse.bass_utils import run_bass_kernel_spmd

BIN_RES = 0.01
NUM_BINS = 512
T0 = 0.0
DECAY = 2.0
N = 131072
NCORES = 8
P = 128                    # partitions
NPC = N // NCORES          # points per core
TILES = NPC // P           # 128 point-tiles per core
HB = BIN_RES / 2.0
C1 = float(np.sqrt(0.5 / np.pi))


def _build(spx, spy, spz):
    nc = bacc.Bacc(None, target_bir_lowering=False)
    f32 = mybir.dt.float32
    AF = mybir.ActivationFunctionType
    OP = mybir.AluOpType

    # packed per-point inputs: blocks mx,my,mz,col,cf,op,pas each [128, TILES]
    pk = nc.dram_tensor("pk", [P, 7 * TILES], f32, kind="ExternalInput")
    rfull = nc.dram_tensor("rfull", [P, NUM_BINS], f32, kind="ExternalInput")
    ird = nc.dram_tensor("ird", [1, NUM_BINS], f32, kind="ExternalInput")
    hist = nc.dram_tensor("hist", [1, NUM_BINS], f32, kind="ExternalOutput")

    with TileContext(nc) as tc:
        with tc.tile_pool(name="const", bufs=1) as const, \
             tc.tile_pool(name="work", bufs=3) as work, \
             tc.tile_pool(name="psum", bufs=1, space="PSUM") as psum:
            pkt = const.tile([P, 7 * TILES], f32)
            nc.gpsimd.dma_start(out=pkt, in_=pk[:, :])
            rt = const.tile([P, NUM_BINS], f32)
            nc.gpsimd.dma_start(out=rt, in_=rfull[:, :])
            irdt = const.tile([1, NUM_BINS], f32)
            nc.gpsimd.dma_start(out=irdt, in_=ird[:, :])

            T = TILES
            mx = pkt[:, 0 * T:1 * T]
            my = pkt[:, 1 * T:2 * T]
            mz = pkt[:, 2 * T:3 * T]
            col = pkt[:, 3 * T:4 * T]
            cf = pkt[:, 4 * T:5 * T]
            opa = pkt[:, 5 * T:6 * T]
            pas = pkt[:, 6 * T:7 * T]

            # ---- per-point prep ([128, T] tiles) ----
            spc = const.tile([P, 4], f32)
            nc.vector.memset(spc[:, 0:1], -spx)
            nc.vector.memset(spc[:, 1:2], -spy)
            nc.vector.memset(spc[:, 2:3], -spz)
            nc.vector.memset(spc[:, 3:4], 1e-12)
            dx2 = const.tile([P, T], f32)
            nc.scalar.activation(out=dx2, in_=mx, func=AF.Square, bias=spc[:, 0:1])
            dy2 = const.tile([P, T], f32)
            nc.scalar.activation(out=dy2, in_=my, func=AF.Square, bias=spc[:, 1:2])
            dz2 = const.tile([P, T], f32)
            nc.scalar.activation(out=dz2, in_=mz, func=AF.Square, bias=spc[:, 2:3])
            r0sq = const.tile([P, T], f32)
            nc.vector.tensor_tensor(out=r0sq, in0=dx2, in1=dy2, op=OP.add)
            nc.vector.tensor_tensor(out=r0sq, in0=r0sq, in1=dz2, op=OP.add)
            # r0 = exp(0.5*ln(r0sq))  (sqrt via ln/exp: same ACT table set)
            lnr = const.tile([P, T], f32)
            nc.scalar.activation(out=lnr, in_=r0sq, func=AF.Ln, bias=spc[:, 3:4])
            r0 = const.tile([P, T], f32)
            nc.scalar.activation(out=r0, in_=lnr, func=AF.Exp, scale=0.5)

            # inv_sigma (clipped): min(exp(-pas), 1/HB); sigma_c = max(exp(pas), HB)
            isig = const.tile([P, T], f32)
            nc.scalar.activation(out=isig, in_=pas, func=AF.Exp, scale=-1.0)
            nc.vector.tensor_scalar(out=isig, in0=isig, scalar1=1.0 / HB,
                                    scalar2=None, op0=OP.min)
            sig = const.tile([P, T], f32)
            nc.scalar.activation(out=sig, in_=pas, func=AF.Exp)
            nc.vector.tensor_scalar(out=sig, in0=sig, scalar1=HB,
                                    scalar2=None, op0=OP.max)

            # coeff = sigmoid(cf) = 1/(1+exp(-cf)); om = 1-coeff = 1/(1+exp(cf))
            # compute om directly, and coeff = 1 - om
            om = const.tile([P, T], f32)
            nc.scalar.activation(out=om, in_=cf, func=AF.Exp)
            nc.vector.tensor_scalar(out=om, in0=om, scalar1=1.0,
                                    scalar2=None, op0=OP.add)
            nc.vector.reciprocal(out=om, in_=om)
            coeff = const.tile([P, T], f32)
            nc.vector.tensor_scalar(out=coeff, in0=om, scalar1=-1.0, scalar2=1.0,
                                    op0=OP.mult, op1=OP.add)

            # I = sigmoid(opa) * col^2
            osig = const.tile([P, T], f32)
            nc.scalar.activation(out=osig, in_=opa, func=AF.Exp, scale=-1.0)
            nc.vector.tensor_scalar(out=osig, in0=osig, scalar1=1.0,
                                    scalar2=None, op0=OP.add)
            nc.vector.reciprocal(out=osig, in_=osig)
            col2 = const.tile([P, T], f32)
            nc.vector.tensor_tensor(out=col2, in0=col, in1=col, op=OP.mult)
            inten = const.tile([P, T], f32)
            nc.vector.tensor_tensor(out=inten, in0=osig, in1=col2, op=OP.mult)

            # ACT scale s = inv_sigma/sqrt(2); bias = -s*r0
            s_all = const.tile([P, T], f32)
            nc.vector.tensor_scalar(out=s_all, in0=isig,
                                    scalar1=float(1.0 / np.sqrt(2.0)),
                                    scalar2=None, op0=OP.mult)
            b_all = const.tile([P, T], f32)
            nc.vector.tensor_tensor(out=b_all, in0=s_all, in1=r0, op=OP.mult)
            nc.vector.tensor_scalar(out=b_all, in0=b_all, scalar1=-1.0,
                                    scalar2=None, op0=OP.mult)

            # gamma = coeff*c1*sigma_c/(1-coeff); negthr = gamma - r0
            rec_om = const.tile([P, T], f32)
            nc.vector.reciprocal(out=rec_om, in_=om)
            gam = const.tile([P, T], f32)
            nc.vector.tensor_tensor(out=gam, in0=coeff, in1=sig, op=OP.mult)
            nc.vector.tensor_scalar(out=gam, in0=gam, scalar1=C1,
                                    scalar2=None, op0=OP.mult)
            nc.vector.tensor_tensor(out=gam, in0=gam, in1=rec_om, op=OP.mult)
            negthr = const.tile([P, T], f32)
            nc.vector.tensor_tensor(out=negthr, in0=gam, in1=r0, op=OP.subtract)

            # w = I * beta * sqrt(pi)/2, beta = HB*(1-coeff)*inv_sigma^2
            isq = const.tile([P, T], f32)
            nc.vector.tensor_tensor(out=isq, in0=isig, in1=isig, op=OP.mult)
            w_all = const.tile([P, T], f32)
            nc.vector.tensor_tensor(out=w_all, in0=om, in1=isq, op=OP.mult)
            nc.vector.tensor_tensor(out=w_all, in0=w_all, in1=inten, op=OP.mult)
            nc.vector.tensor_scalar(out=w_all, in0=w_all,
                                    scalar1=float(HB * np.sqrt(np.pi) / 2.0),
                                    scalar2=None, op0=OP.mult)

            # ---- main loop over point-tiles ----
            ps = psum.tile([1, NUM_BINS], f32)
            for t in range(TILES):
                gt = work.tile([P, NUM_BINS], f32, tag="g")
                nc.scalar.activation(
                    out=gt, in_=rt, func=AF.Derivative_Erf,
                    bias=b_all[:, t:t + 1], scale=s_all[:, t:t + 1])
                ht = work.tile([P, NUM_BINS], f32, tag="h")
                nc.vector.tensor_scalar(
                    out=ht, in0=rt, scalar1=negthr[:, t:t + 1], scalar2=0.0,
                    op0=OP.add, op1=OP.max)
                pp = work.tile([P, NUM_BINS], f32, tag="pp")
                nc.vector.tensor_tensor(out=pp, in0=gt, in1=ht, op=OP.mult)
                nc.tensor.matmul(ps, lhsT=w_all[:, t:t + 1], rhs=pp,
                                 start=(t == 0), stop=(t == TILES - 1))

            # hist_partial = ps * r^-DECAY
            hs = const.tile([1, NUM_BINS], f32)
            nc.scalar.copy(out=hs, in_=ps)
            nc.vector.tensor_tensor(out=hs, in0=hs, in1=irdt[0:1, :], op=OP.mult)
            nc.sync.dma_start(out=hist[0:1, :], in_=hs)

    nc.compile()
    return nc


def _shard(inputs):
    means = np.asarray(inputs["means"], dtype=np.float32)
    vid = int(np.asarray(inputs.get("view_id", 0)))
    colours = np.asarray(inputs["colours"], dtype=np.float32)
    coefficients = np.asarray(inputs["coefficients"], dtype=np.float32)
    opacities = np.asarray(inputs["opacities"], dtype=np.float32)
    pre_act_scales = np.asarray(inputs["pre_act_scales"], dtype=np.float32)

    # bin centers r_ and 1/r^DECAY (f32, matching the reference's fp32 math)
    r_ = (np.float32(T0 / 2.0)
          + np.float32(HB) * np.arange(1, 1 + NUM_BINS, dtype=np.float32))
    rd = np.power(r_, np.float32(DECAY), dtype=np.float32)
    ird = (np.float32(1.0) / rd).reshape(1, NUM_BINS)
    rfull = np.broadcast_to(r_, (P, NUM_BINS)).copy()

    def blk(arr, c):
        # core c's slice -> [128 partitions, TILES] with point p = t*128+i
        return np.ascontiguousarray(
            arr[c * NPC:(c + 1) * NPC].reshape(TILES, P).T)

    sig_col = opacities[:, vid]
    in_maps = []
    for c in range(NCORES):
        pk = np.concatenate([
            blk(means[:, 0], c), blk(means[:, 1], c), blk(means[:, 2], c),
            blk(colours[:, 0], c), blk(coefficients[:, 0], c),
            blk(sig_col, c), blk(pre_act_scales[:, 0], c)], axis=1)
        in_maps.append({
            "pk": np.ascontiguousarray(pk, dtype=np.float32),
            "rfull": rfull.astype(np.float32),
            "ird": ird.astype(np.float32),
        })
    return in_maps


def kernel(means, scan_point, colours, coefficients, opacities, pre_act_scales,
           view_id=0, **_unused):
    scan_point = np.asarray(scan_point, dtype=np.float32)
    spx, spy, spz = (float(scan_point[i]) for i in range(3))
    nc = _build(spx, spy, spz)
    in_maps = _shard(dict(means=means, colours=colours,
                          coefficients=coefficients, opacities=opacities,
                          pre_act_scales=pre_act_scales, view_id=view_id))

    res = run_bass_kernel_spmd(nc, in_maps, core_ids=list(range(NCORES)))
    total = np.zeros(NUM_BINS, dtype=np.float64)
    for om in res.results:
        total += om["hist"][0].astype(np.float64)
    return total.astype(np.float32)


# revision 5
# speedup vs baseline: 1.2877x; 1.2877x over previous
"""Gaussian histogram kernel for TRN2, 8 NeuronCores, data-parallel over points.

Math (per point n, bin b):
  r0 = ||means_n - scan_point||, sigma = max(exp(pas_n), hb), hb = BIN_RES/2
  g = exp(-(r_b - r0)^2 / (2 sigma^2))
  pr = clip(hb * (coeff*pdf1 + (1-coeff)*pdf2), 0, 1)
     = g * beta * relu(r_b - thr)          [upper clip never binds: hb<=sigma]
       beta = hb*(1-coeff)/sigma^2, thr = r0 - coeff*c1*sigma/(1-coeff)
  hist_b = sum_n I_n * pr_{n,b} / r_b^2,   I_n = sigmoid(op_n)*col_n^2

On-chip mapping per core (16384 points = 128 tiles of 128 partitions):
  g~  = DerivativeErf(s*R + bias)  (ScalarE LUT; = 2/sqrt(pi) * g)
  hinge = max(R - thr, 0)          (VectorE dual-op tensor_scalar, 2x fp32)
  pp  = g~ * hinge                 (VectorE tensor_tensor)
  PSUM[1,512] += w^T @ pp          (TensorE, w = I*beta*sqrt(pi)/2)
Partial [512] per core; host sums the 8 partials (the all-reduce/unshard).
"""
import numpy as np

import concourse.bacc as bacc
import concourse.mybir as mybir
from concourse.tile import TileContext
from concourse.bass_utils import run_bass_kernel_spmd

BIN_RES = 0.01
NUM_BINS = 512
T0 = 0.0
DECAY = 2.0
N = 131072
NCORES = 8
P = 128                    # partitions
NPC = N // NCORES          # points per core
TILES = NPC // P           # 128 point-tiles per core
HB = BIN_RES / 2.0
C1 = float(np.sqrt(0.5 / np.pi))


def _build(spx, spy, spz):
    nc = bacc.Bacc(None, target_bir_lowering=False)
    f32 = mybir.dt.float32
    AF = mybir.ActivationFunctionType
    OP = mybir.AluOpType

    # packed per-point inputs: blocks mx,my,mz,col,cf,op,pas each [128, TILES]
    pk = nc.dram_tensor("pk", [P, 7 * TILES], f32, kind="ExternalInput")
    rfull = nc.dram_tensor("rfull", [P, NUM_BINS], f32, kind="ExternalInput")
    ird = nc.dram_tensor("ird", [1, NUM_BINS], f32, kind="ExternalInput")
    hist = nc.dram_tensor("hist", [1, NUM_BINS], f32, kind="ExternalOutput")

    with TileContext(nc) as tc:
        with tc.tile_pool(name="const", bufs=1) as const, \
             tc.tile_pool(name="work", bufs=3) as work, \
             tc.tile_pool(name="psum", bufs=1, space="PSUM") as psum:
            pkt = const.tile([P, 7 * TILES], f32)
            nc.gpsimd.dma_start(out=pkt, in_=pk[:, :])
            rt = const.tile([P, NUM_BINS], f32)
            nc.gpsimd.dma_start(out=rt, in_=rfull[:, :])
            irdt = const.tile([1, NUM_BINS], f32)
            nc.gpsimd.dma_start(out=irdt, in_=ird[:, :])

            T = TILES
            mx = pkt[:, 0 * T:1 * T]
            my = pkt[:, 1 * T:2 * T]
            mz = pkt[:, 2 * T:3 * T]
            col = pkt[:, 3 * T:4 * T]
            cf = pkt[:, 4 * T:5 * T]
            opa = pkt[:, 5 * T:6 * T]
            pas = pkt[:, 6 * T:7 * T]

            # ---- per-point prep ([128, T] tiles) ----
            spc = const.tile([P, 4], f32)
            nc.vector.memset(spc[:, 0:1], -spx)
            nc.vector.memset(spc[:, 1:2], -spy)
            nc.vector.memset(spc[:, 2:3], -spz)
            nc.vector.memset(spc[:, 3:4], 1e-12)
            dx2 = const.tile([P, T], f32)
            nc.scalar.activation(out=dx2, in_=mx, func=AF.Square, bias=spc[:, 0:1])
            dy2 = const.tile([P, T], f32)
            nc.scalar.activation(out=dy2, in_=my, func=AF.Square, bias=spc[:, 1:2])
            dz2 = const.tile([P, T], f32)
            nc.scalar.activation(out=dz2, in_=mz, func=AF.Square, bias=spc[:, 2:3])
            r0sq = const.tile([P, T], f32)
            nc.vector.tensor_tensor(out=r0sq, in0=dx2, in1=dy2, op=OP.add)
            nc.vector.tensor_tensor(out=r0sq, in0=r0sq, in1=dz2, op=OP.add)
            # r0 = exp(0.5*ln(r0sq))  (sqrt via ln/exp: same ACT table set)
            lnr = const.tile([P, T], f32)
            nc.scalar.activation(out=lnr, in_=r0sq, func=AF.Ln, bias=spc[:, 3:4])
            r0 = const.tile([P, T], f32)
            nc.scalar.activation(out=r0, in_=lnr, func=AF.Exp, scale=0.5)

            # inv_sigma (clipped): min(exp(-pas), 1/HB); sigma_c = max(exp(pas), HB)
            isig = const.tile([P, T], f32)
            nc.scalar.activation(out=isig, in_=pas, func=AF.Exp, scale=-1.0)
            nc.vector.tensor_scalar(out=isig, in0=isig, scalar1=1.0 / HB,
                                    scalar2=None, op0=OP.min)
            sig = const.tile([P, T], f32)
            nc.scalar.activation(out=sig, in_=pas, func=AF.Exp)
            nc.vector.tensor_scalar(out=sig, in0=sig, scalar1=HB,
                                    scalar2=None, op0=OP.max)

            # coeff = sigmoid(cf) = 1/(1+exp(-cf)); om = 1-coeff = 1/(1+exp(cf))
            # compute om directly, and coeff = 1 - om
            om = const.tile([P, T], f32)
            nc.scalar.activation(out=om, in_=cf, func=AF.Exp)
            nc.vector.tensor_scalar(out=om, in0=om, scalar1=1.0,
                                    scalar2=None, op0=OP.add)
            nc.vector.reciprocal(out=om, in_=om)
            coeff = const.tile([P, T], f32)
            nc.vector.tensor_scalar(out=coeff, in0=om, scalar1=-1.0, scalar2=1.0,
                                    op0=OP.mult, op1=OP.add)

            # I = sigmoid(opa) * col^2
            osig = const.tile([P, T], f32)
            nc.scalar.activation(out=osig, in_=opa, func=AF.Exp, scale=-1.0)
            nc.vector.tensor_scalar(out=osig, in0=osig, scalar1=1.0,
                                    scalar2=None, op0=OP.add)
            nc.vector.reciprocal(out=osig, in_=osig)
            col2 = const.tile([P, T], f32)
            nc.vector.tensor_tensor(out=col2, in0=col, in1=col, op=OP.mult)
            inten = const.tile([P, T], f32)
            nc.vector.tensor_tensor(out=inten, in0=osig, in1=col2, op=OP.mult)

            # ACT scale s = inv_sigma/sqrt(2); bias = -s*r0
            s_all = const.tile([P, T], f32)
            nc.vector.tensor_scalar(out=s_all, in0=isig,
                                    scalar1=float(1.0 / np.sqrt(2.0)),
                                    scalar2=None, op0=OP.mult)
            b_all = const.tile([P, T], f32)
            nc.vector.tensor_tensor(out=b_all, in0=s_all, in1=r0, op=OP.mult)
            nc.vector.tensor_scalar(out=b_all, in0=b_all, scalar1=-1.0,
                                    scalar2=None, op0=OP.mult)

            # gamma = coeff*c1*sigma_c/(1-coeff); negthr = gamma - r0
            rec_om = const.tile([P, T], f32)
            nc.vector.reciprocal(out=rec_om, in_=om)
            gam = const.tile([P, T], f32)
            nc.vector.tensor_tensor(out=gam, in0=coeff, in1=sig, op=OP.mult)
            nc.vector.tensor_scalar(out=gam, in0=gam, scalar1=C1,
                                    scalar2=None, op0=OP.mult)
            nc.vector.tensor_tensor(out=gam, in0=gam, in1=rec_om, op=OP.mult)
            negthr = const.tile([P, T], f32)
            nc.vector.tensor_tensor(out=negthr, in0=gam, in1=r0, op=OP.subtract)

            # w = I * beta * sqrt(pi)/2, beta = HB*(1-coeff)*inv_sigma^2
            isq = const.tile([P, T], f32)
            nc.vector.tensor_tensor(out=isq, in0=isig, in1=isig, op=OP.mult)
            w_all = const.tile([P, T], f32)
            nc.vector.tensor_tensor(out=w_all, in0=om, in1=isq, op=OP.mult)
            nc.vector.tensor_tensor(out=w_all, in0=w_all, in1=inten, op=OP.mult)
            nc.vector.tensor_scalar(out=w_all, in0=w_all,
                                    scalar1=float(HB * np.sqrt(np.pi) / 2.0),
                                    scalar2=None, op0=OP.mult)
            w_bf = const.tile([P, T], mybir.dt.bfloat16)
            nc.vector.tensor_copy(out=w_bf, in_=w_all)

            # ---- main loop over point-tiles ----
            ps = psum.tile([1, NUM_BINS], f32)
            for t in range(TILES):
                gt = work.tile([P, NUM_BINS], mybir.dt.bfloat16, tag="g")
                nc.scalar.activation(
                    out=gt, in_=rt, func=AF.Derivative_Erf,
                    bias=b_all[:, t:t + 1], scale=s_all[:, t:t + 1])
                ht = work.tile([P, NUM_BINS], mybir.dt.bfloat16, tag="h")
                nc.vector.tensor_scalar(
                    out=ht, in0=rt, scalar1=negthr[:, t:t + 1], scalar2=0.0,
                    op0=OP.add, op1=OP.max)
                pp = work.tile([P, NUM_BINS], mybir.dt.bfloat16, tag="pp")
                nc.vector.tensor_tensor(out=pp, in0=gt, in1=ht, op=OP.mult)
                nc.tensor.matmul(ps, lhsT=w_bf[:, t:t + 1], rhs=pp,
                                 start=(t == 0), stop=(t == TILES - 1))

            # hist_partial = ps * r^-DECAY
            hs = const.tile([1, NUM_BINS], f32)
            nc.scalar.copy(out=hs, in_=ps)
            nc.vector.tensor_tensor(out=hs, in0=hs, in1=irdt[0:1, :], op=OP.mult)
            nc.sync.dma_start(out=hist[0:1, :], in_=hs)

    nc.compile()
    return nc


def _shard(inputs):
    means = np.asarray(inputs["means"], dtype=np.float32)
    vid = int(np.asarray(inputs.get("view_id", 0)))
    colours = np.asarray(inputs["colours"], dtype=np.float32)
    coefficients = np.asarray(inputs["coefficients"], dtype=np.float32)
    opacities = np.asarray(inputs["opacities"], dtype=np.float32)
    pre_act_scales = np.asarray(inputs["pre_act_scales"], dtype=np.float32)

    # bin centers r_ and 1/r^DECAY (f32, matching the reference's fp32 math)
    r_ = (np.float32(T0 / 2.0)
          + np.float32(HB) * np.arange(1, 1 + NUM_BINS, dtype=np.float32))
    rd = np.power(r_, np.float32(DECAY), dtype=np.float32)
    ird = (np.float32(1.0) / rd).reshape(1, NUM_BINS)
    rfull = np.broadcast_to(r_, (P, NUM_BINS)).copy()

    def blk(arr, c):
        # core c's slice -> [128 partitions, TILES] with point p = t*128+i
        return np.ascontiguousarray(
            arr[c * NPC:(c + 1) * NPC].reshape(TILES, P).T)

    sig_col = opacities[:, vid]
    in_maps = []
    for c in range(NCORES):
        pk = np.concatenate([
            blk(means[:, 0], c), blk(means[:, 1], c), blk(means[:, 2], c),
            blk(colours[:, 0], c), blk(coefficients[:, 0], c),
            blk(sig_col, c), blk(pre_act_scales[:, 0], c)], axis=1)
        in_maps.append({
            "pk": np.ascontiguousarray(pk, dtype=np.float32),
            "rfull": rfull.astype(np.float32),
            "ird": ird.astype(np.float32),
        })
    return in_maps


def kernel(means, scan_point, colours, coefficients, opacities, pre_act_scales,
           view_id=0, **_unused):
    scan_point = np.asarray(scan_point, dtype=np.float32)
    spx, spy, spz = (float(scan_point[i]) for i in range(3))
    nc = _build(spx, spy, spz)
    in_maps = _shard(dict(means=means, colours=colours,
                          coefficients=coefficients, opacities=opacities,
                          pre_act_scales=pre_act_scales, view_id=view_id))

    res = run_bass_kernel_spmd(nc, in_maps, core_ids=list(range(NCORES)))
    total = np.zeros(NUM_BINS, dtype=np.float64)
    for om in res.results:
        total += om["hist"][0].astype(np.float64)
    return total.astype(np.float32)


# revision 7
# speedup vs baseline: 1.3072x; 1.0152x over previous
"""Gaussian histogram kernel for TRN2, 8 NeuronCores, data-parallel over points.

Math (per point n, bin b):
  r0 = ||means_n - scan_point||, sigma = max(exp(pas_n), hb), hb = BIN_RES/2
  g = exp(-(r_b - r0)^2 / (2 sigma^2))
  pr = clip(hb * (coeff*pdf1 + (1-coeff)*pdf2), 0, 1)
     = g * beta * relu(r_b - thr)          [upper clip never binds: hb<=sigma]
       beta = hb*(1-coeff)/sigma^2, thr = r0 - coeff*c1*sigma/(1-coeff)
  hist_b = sum_n I_n * pr_{n,b} / r_b^2,   I_n = sigmoid(op_n)*col_n^2

On-chip mapping per core (16384 points = 128 tiles of 128 partitions):
  g~  = DerivativeErf(s*R + bias)  (ScalarE LUT; = 2/sqrt(pi) * g)
  hinge = max(R - thr, 0)          (VectorE dual-op tensor_scalar, 2x fp32)
  pp  = g~ * hinge                 (VectorE tensor_tensor)
  PSUM[1,512] += w^T @ pp          (TensorE, w = I*beta*sqrt(pi)/2)
Partial [512] per core; host sums the 8 partials (the all-reduce/unshard).
"""
import numpy as np

import concourse.bacc as bacc
import concourse.mybir as mybir
from concourse.tile import TileContext
from concourse.bass_utils import run_bass_kernel_spmd

BIN_RES = 0.01
NUM_BINS = 512
T0 = 0.0
DECAY = 2.0
N = 131072
NCORES = 8
P = 128                    # partitions
NPC = N // NCORES          # points per core
TILES = NPC // P           # 128 point-tiles per core
HB = BIN_RES / 2.0
C1 = float(np.sqrt(0.5 / np.pi))


def _build(spx, spy, spz):
    nc = bacc.Bacc(None, target_bir_lowering=False)
    f32 = mybir.dt.float32
    AF = mybir.ActivationFunctionType
    OP = mybir.AluOpType

    # packed per-point inputs: blocks mx,my,mz,col,cf,op,pas each [128, TILES]
    pk = nc.dram_tensor("pk", [P, 7 * TILES], f32, kind="ExternalInput")
    rfull = nc.dram_tensor("rfull", [P, NUM_BINS], f32, kind="ExternalInput")
    ird = nc.dram_tensor("ird", [1, NUM_BINS], f32, kind="ExternalInput")
    hist = nc.dram_tensor("hist", [1, NUM_BINS], f32, kind="ExternalOutput")

    with TileContext(nc) as tc:
        with tc.tile_pool(name="const", bufs=1) as const, \
             tc.tile_pool(name="work", bufs=3) as work, \
             tc.tile_pool(name="psum", bufs=1, space="PSUM") as psum:
            pkt = const.tile([P, 7 * TILES], f32)
            nc.gpsimd.dma_start(out=pkt, in_=pk[:, :])
            rt = const.tile([P, NUM_BINS], f32)
            nc.gpsimd.dma_start(out=rt, in_=rfull[:, :])
            irdt = const.tile([1, NUM_BINS], f32)
            nc.gpsimd.dma_start(out=irdt, in_=ird[:, :])

            T = TILES
            mx = pkt[:, 0 * T:1 * T]
            my = pkt[:, 1 * T:2 * T]
            mz = pkt[:, 2 * T:3 * T]
            col = pkt[:, 3 * T:4 * T]
            cf = pkt[:, 4 * T:5 * T]
            opa = pkt[:, 5 * T:6 * T]
            pas = pkt[:, 6 * T:7 * T]

            # ---- per-point prep ([128, T] tiles) ----
            spc = const.tile([P, 4], f32)
            nc.vector.memset(spc[:, 0:1], -spx)
            nc.vector.memset(spc[:, 1:2], -spy)
            nc.vector.memset(spc[:, 2:3], -spz)
            nc.vector.memset(spc[:, 3:4], 1e-12)
            dx2 = const.tile([P, T], f32)
            nc.scalar.activation(out=dx2, in_=mx, func=AF.Square, bias=spc[:, 0:1])
            dy2 = const.tile([P, T], f32)
            nc.scalar.activation(out=dy2, in_=my, func=AF.Square, bias=spc[:, 1:2])
            dz2 = const.tile([P, T], f32)
            nc.scalar.activation(out=dz2, in_=mz, func=AF.Square, bias=spc[:, 2:3])
            r0sq = const.tile([P, T], f32)
            nc.vector.tensor_tensor(out=r0sq, in0=dx2, in1=dy2, op=OP.add)
            nc.vector.tensor_tensor(out=r0sq, in0=r0sq, in1=dz2, op=OP.add)
            # r0 = exp(0.5*ln(r0sq))  (sqrt via ln/exp: same ACT table set)
            lnr = const.tile([P, T], f32)
            nc.scalar.activation(out=lnr, in_=r0sq, func=AF.Ln, bias=spc[:, 3:4])
            r0 = const.tile([P, T], f32)
            nc.scalar.activation(out=r0, in_=lnr, func=AF.Exp, scale=0.5)

            # inv_sigma (clipped): min(exp(-pas), 1/HB); sigma_c = max(exp(pas), HB)
            isig = const.tile([P, T], f32)
            nc.scalar.activation(out=isig, in_=pas, func=AF.Exp, scale=-1.0)
            nc.vector.tensor_scalar(out=isig, in0=isig, scalar1=1.0 / HB,
                                    scalar2=None, op0=OP.min)
            sig = const.tile([P, T], f32)
            nc.scalar.activation(out=sig, in_=pas, func=AF.Exp)
            nc.vector.tensor_scalar(out=sig, in0=sig, scalar1=HB,
                                    scalar2=None, op0=OP.max)

            # om = 1-coeff = 1/(1+exp(cf)); coeff/(1-coeff) = exp(cf) directly
            ecf = const.tile([P, T], f32)
            nc.scalar.activation(out=ecf, in_=cf, func=AF.Exp)
            om = const.tile([P, T], f32)
            nc.vector.tensor_scalar(out=om, in0=ecf, scalar1=1.0,
                                    scalar2=None, op0=OP.add)
            nc.vector.reciprocal(out=om, in_=om)

            # I = sigmoid(opa) * col^2
            osig = const.tile([P, T], f32)
            nc.scalar.activation(out=osig, in_=opa, func=AF.Exp, scale=-1.0)
            nc.vector.tensor_scalar(out=osig, in0=osig, scalar1=1.0,
                                    scalar2=None, op0=OP.add)
            nc.vector.reciprocal(out=osig, in_=osig)
            col2 = const.tile([P, T], f32)
            nc.vector.tensor_tensor(out=col2, in0=col, in1=col, op=OP.mult)
            inten = const.tile([P, T], f32)
            nc.vector.tensor_tensor(out=inten, in0=osig, in1=col2, op=OP.mult)

            # ACT scale s = inv_sigma/sqrt(2); bias = -s*r0
            s_all = const.tile([P, T], f32)
            nc.vector.tensor_scalar(out=s_all, in0=isig,
                                    scalar1=float(1.0 / np.sqrt(2.0)),
                                    scalar2=None, op0=OP.mult)
            b_all = const.tile([P, T], f32)
            nc.vector.tensor_tensor(out=b_all, in0=s_all, in1=r0, op=OP.mult)
            nc.vector.tensor_scalar(out=b_all, in0=b_all, scalar1=-1.0,
                                    scalar2=None, op0=OP.mult)

            # gamma = c1*sigma_c*exp(cf)  [= coeff*c1*sigma/(1-coeff)]
            gam = const.tile([P, T], f32)
            nc.vector.tensor_tensor(out=gam, in0=ecf, in1=sig, op=OP.mult)
            nc.vector.tensor_scalar(out=gam, in0=gam, scalar1=C1,
                                    scalar2=None, op0=OP.mult)
            negthr = const.tile([P, T], f32)
            nc.vector.tensor_tensor(out=negthr, in0=gam, in1=r0, op=OP.subtract)

            # w = I * beta * sqrt(pi)/2, beta = HB*(1-coeff)*inv_sigma^2
            isq = const.tile([P, T], f32)
            nc.vector.tensor_tensor(out=isq, in0=isig, in1=isig, op=OP.mult)
            w_all = const.tile([P, T], f32)
            nc.vector.tensor_tensor(out=w_all, in0=om, in1=isq, op=OP.mult)
            nc.vector.tensor_tensor(out=w_all, in0=w_all, in1=inten, op=OP.mult)
            nc.vector.tensor_scalar(out=w_all, in0=w_all,
                                    scalar1=float(HB * np.sqrt(np.pi) / 2.0),
                                    scalar2=None, op0=OP.mult)
            w_bf = const.tile([P, T], mybir.dt.bfloat16)
            nc.vector.tensor_copy(out=w_bf, in_=w_all)

            # ---- main loop over point-tiles ----
            ps = psum.tile([1, NUM_BINS], f32)
            for t in range(TILES):
                gt = work.tile([P, NUM_BINS], mybir.dt.bfloat16, tag="g")
                nc.scalar.activation(
                    out=gt, in_=rt, func=AF.Derivative_Erf,
                    bias=b_all[:, t:t + 1], scale=s_all[:, t:t + 1])
                ht = work.tile([P, NUM_BINS], mybir.dt.bfloat16, tag="h")
                nc.vector.tensor_scalar(
                    out=ht, in0=rt, scalar1=negthr[:, t:t + 1], scalar2=0.0,
                    op0=OP.add, op1=OP.max)
                pp = work.tile([P, NUM_BINS], mybir.dt.bfloat16, tag="pp")
                nc.vector.tensor_tensor(out=pp, in0=gt, in1=ht, op=OP.mult)
                nc.tensor.matmul(ps, lhsT=w_bf[:, t:t + 1], rhs=pp,
                                 start=(t == 0), stop=(t == TILES - 1))

            # hist_partial = ps * r^-DECAY
            hs = const.tile([1, NUM_BINS], f32)
            nc.scalar.copy(out=hs, in_=ps)
            nc.vector.tensor_tensor(out=hs, in0=hs, in1=irdt[0:1, :], op=OP.mult)
            nc.sync.dma_start(out=hist[0:1, :], in_=hs)

    nc.compile()
    return nc


def _shard(inputs):
    means = np.asarray(inputs["means"], dtype=np.float32)
    vid = int(np.asarray(inputs.get("view_id", 0)))
    colours = np.asarray(inputs["colours"], dtype=np.float32)
    coefficients = np.asarray(inputs["coefficients"], dtype=np.float32)
    opacities = np.asarray(inputs["opacities"], dtype=np.float32)
    pre_act_scales = np.asarray(inputs["pre_act_scales"], dtype=np.float32)

    # bin centers r_ and 1/r^DECAY (f32, matching the reference's fp32 math)
    r_ = (np.float32(T0 / 2.0)
          + np.float32(HB) * np.arange(1, 1 + NUM_BINS, dtype=np.float32))
    rd = np.power(r_, np.float32(DECAY), dtype=np.float32)
    ird = (np.float32(1.0) / rd).reshape(1, NUM_BINS)
    rfull = np.broadcast_to(r_, (P, NUM_BINS)).copy()

    def blk(arr, c):
        # core c's slice -> [128 partitions, TILES] with point p = t*128+i
        return np.ascontiguousarray(
            arr[c * NPC:(c + 1) * NPC].reshape(TILES, P).T)

    sig_col = opacities[:, vid]
    in_maps = []
    for c in range(NCORES):
        pk = np.concatenate([
            blk(means[:, 0], c), blk(means[:, 1], c), blk(means[:, 2], c),
            blk(colours[:, 0], c), blk(coefficients[:, 0], c),
            blk(sig_col, c), blk(pre_act_scales[:, 0], c)], axis=1)
        in_maps.append({
            "pk": np.ascontiguousarray(pk, dtype=np.float32),
            "rfull": rfull.astype(np.float32),
            "ird": ird.astype(np.float32),
        })
    return in_maps


def kernel(means, scan_point, colours, coefficients, opacities, pre_act_scales,
           view_id=0, **_unused):
    scan_point = np.asarray(scan_point, dtype=np.float32)
    spx, spy, spz = (float(scan_point[i]) for i in range(3))
    nc = _build(spx, spy, spz)
    in_maps = _shard(dict(means=means, colours=colours,
                          coefficients=coefficients, opacities=opacities,
                          pre_act_scales=pre_act_scales, view_id=view_id))

    res = run_bass_kernel_spmd(nc, in_maps, core_ids=list(range(NCORES)))
    total = np.zeros(NUM_BINS, dtype=np.float64)
    for om in res.results:
        total += om["hist"][0].astype(np.float64)
    return total.astype(np.float32)
